# revision 16
# baseline (speedup 1.0000x reference)
"""Trainium2 Bass kernel: batched multi-head self-attention (nn_Attention).

y = softmax(q k^T / sqrt(64)) v, projected; x (8, 1025, 768), 12 heads x 64.

Strategy: batch-parallel across the 8 NeuronCores (one batch element per
core, no collectives). Per core, everything is kept feature-major
(transposed) so no on-chip transposes are needed:
  qkT = wqkT.T @ xT;  v = xT.T @ wvT (with a per-head ones column);
  scoresT = kT.T @ qT (keys on partitions, head pairs row-packed in the PE
  array);  exp on the scalar engine;  [v|1](128-wide).T @ attnT accumulated
  over key tiles yields the weighted values AND the softmax denominator in
  one PSUM accumulation.

Pipeline: v-projection first (c-outer so the PE starts as soon as the first
x/wv chunks land), then a fused per-head-pair loop [qk-projection for the
pair -> attention for the pair] so the scalar engine's exp stream overlaps
the projection matmuls instead of idling through a separate projection
phase. Normalization is deferred: numerators parked in SBUF, denominators
DMA-gathered onto 12 partitions, ONE batched reciprocal per query chunk
(DVE cost is free-size-only), reciprocal rows DMA-scattered and broadcast,
then multiplied into aoT. Out-projection (+bias on the scalar engine)
drains at the end.

Operands are fp16 (inputs/weights/q/k/v, ~2e-3 relative accuracy) except the
exp'd attention weights, which are bf16 (exp reaches ~5e6, beyond fp16
range); all accumulation is fp32 in PSUM. Full-array "warmer" matmuls keep
the PE HAM clock at 8/8 through the partial-array attention matmuls.
"""
import sys

try:
    import concourse.bass  # noqa: F401
except ImportError:
    sys.path.insert(0, "/opt/trn_rl_repo")

import numpy as np

from contextlib import ExitStack

import concourse.bass as bass
import concourse.tile as tile
from concourse import bacc, mybir

F32 = mybir.dt.float32
F32R = mybir.dt.float32r
BF16 = mybir.dt.bfloat16
F16 = mybir.dt.float16

C = 768
H = 12
D = 64
NTOK = 1025
T = 1032
CT = C // 128
SCALE = D ** -0.5

KT = [(i * 128, 128) for i in range(8)] + [(1024, 8)]
QC = [(0, 512), (512, 512)]
QTAIL = (1024, 8)
SC_GROUPS = [(0, 1), (2, 3), (4, 5), (6, 7), (8,)]
VW = 65


def build(matmul_dtype="fp16"):
    if matmul_dtype == "fp16":
        MT = AT = F16
        ATTN = BF16
    elif matmul_dtype == "bf16":
        MT = AT = ATTN = BF16
    else:
        MT = AT = ATTN = F32
    nc = bacc.Bacc("TRN2", target_bir_lowering=False, debug=False, num_devices=8)

    xT_d = nc.dram_tensor("xT", [C, T], MT, kind="ExternalInput")
    wqkT_d = nc.dram_tensor("wqkT", [C, 2 * C], MT, kind="ExternalInput")
    wvT_d = nc.dram_tensor("wvT", [C, C], MT, kind="ExternalInput")
    wpT_d = nc.dram_tensor("wpT", [C, C], MT, kind="ExternalInput")
    bp_d = nc.dram_tensor("bp", [C, 1], F32, kind="ExternalInput")
    yT_d = nc.dram_tensor("yT", [C, T], F16 if matmul_dtype == "fp16" else F32,
                          kind="ExternalOutput")

    with tile.TileContext(nc) as tc, ExitStack() as ctx:
        p_qk = ctx.enter_context(tc.tile_pool(name="qk", bufs=1))
        p_v = ctx.enter_context(tc.tile_pool(name="v", bufs=1))
        p_ao = ctx.enter_context(tc.tile_pool(name="ao", bufs=1))
        p_bp = ctx.enter_context(tc.tile_pool(name="bp", bufs=1))
        p_attn = ctx.enter_context(tc.tile_pool(name="attn", bufs=1))
        p_sm = ctx.enter_context(tc.tile_pool(name="sm", bufs=1))
        p_stage = ctx.enter_context(tc.tile_pool(name="stage", bufs=4))
        p_w = ctx.enter_context(tc.tile_pool(name="w", bufs=1))

        qkT = [p_qk.tile([128, T], AT, tag=f"qkT{i}", name=f"qkT{i}")
               for i in range(12)]
        v_ext = [p_v.tile([128, H * VW + 63], AT, tag=f"v{i}", name=f"v{i}")
                 for i in range(9)]
        aoT = [p_ao.tile([128, T], MT, tag=f"ao{i}", name=f"ao{i}")
               for i in range(CT)]
        bp_sb = [p_bp.tile([128, 1], F32, tag=f"bp{i}", name=f"bp{i}")
                 for i in range(CT)]

        xT = [p_w.tile([128, T], MT, tag=f"x{i}", name=f"x{i}") for i in range(CT)]
        wvT = [p_w.tile([128, C], MT, tag=f"wv{i}", name=f"wv{i}")
               for i in range(CT)]
        wqk = [p_w.tile([128, 2 * C], MT, tag=f"wqk{i}", name=f"wqk{i}")
               for i in range(CT)]
        wp = [p_w.tile([128, C], MT, tag=f"wp{i}", name=f"wp{i}")
              for i in range(CT)]
        for c in range(CT):
            nc.sync.dma_start(xT[c][:], xT_d.ap()[c * 128:(c + 1) * 128, :])
            nc.sync.dma_start(wvT[c][:], wvT_d.ap()[c * 128:(c + 1) * 128, :])
        for c in range(CT):
            nc.sync.dma_start(wqk[c][:], wqkT_d.ap()[c * 128:(c + 1) * 128, :])
        for c in range(CT):
            nc.sync.dma_start(wp[c][:], wpT_d.ap()[c * 128:(c + 1) * 128, :])
            nc.sync.dma_start(bp_sb[c][:], bp_d.ap()[c * 128:(c + 1) * 128, :])

        # ---- v projection: c-outer in nt-blocks of 4 so the first matmul
        # only waits on the first x/wv chunk DMA ----
        with nc.named_scope("v_proj"), \
             tc.tile_pool(name="psv", bufs=4, space="PSUM") as psv:
            for blk in ((0, 1, 2, 3), (4, 5, 6, 7), (8,)):
                pss = {nt: psv.tile([128, 768], F32, tag="psv", name="psv")
                       for nt in blk}
                for c in range(CT):
                    for nt in blk:
                        noff, nsz = KT[nt]
                        for (voff, vsz) in ((0, 512), (512, 256)):
                            nc.tensor.matmul(
                                pss[nt][:nsz, voff:voff + vsz],
                                xT[c][:, noff:noff + nsz],
                                wvT[c][:, voff:voff + vsz],
                                start=(c == 0), stop=(c == CT - 1),
                            )
                for nt in blk:
                    noff, nsz = KT[nt]
                    for (voff, vsz) in ((0, 512), (512, 256)):
                        nh = vsz // D
                        h0 = voff // D
                        dst = (
                            v_ext[nt][0:nsz, h0 * VW:(h0 + nh) * VW]
                            .rearrange("p (hh w) -> p hh w", w=VW)[:, :, 0:D]
                        )
                        src = pss[nt][0:nsz, voff:voff + vsz].rearrange(
                            "p (hh w) -> p hh w", w=D)
                        nc.vector.tensor_copy(dst, src)
                    if nt < 8:
                        ones_col = (
                            v_ext[nt][0:nsz, 0:H * VW]
                            .rearrange("p (hh w) -> p hh w", w=VW)[:, :, D:VW]
                        )
                        _memset(nc, AT, ones_col, one=True)
                    else:
                        pad_col = (
                            v_ext[nt][0:nsz, 0:H * VW]
                            .rearrange("p (hh w) -> p hh w", w=VW)[:, :, D:VW]
                        )
                        _memset(nc, AT, pad_col, one=False)
                        one_row = (
                            v_ext[nt][0:1, 0:H * VW]
                            .rearrange("p (hh w) -> p hh w", w=VW)[:, :, D:VW]
                        )
                        _memset(nc, AT, one_row, one=True)
            for nt in range(9):
                _memset(nc, AT, v_ext[nt][:, H * VW:H * VW + 63], one=False)

        # ---- fused pair loop: qk-projection for the pair, then attention ----
        with tc.tile_pool(name="psP", bufs=2, space="PSUM") as psP, \
             tc.tile_pool(name="psA", bufs=1, space="PSUM") as psA:

            def scores_mm(sc, pair, h_in_pair, kt, dst_off, qoff, qsz):
                koff, ksz = KT[kt]
                p0 = 64 * h_in_pair
                nc.tensor.matmul(
                    sc[0:ksz, dst_off:dst_off + qsz],
                    qkT[6 + pair][p0:p0 + 64, koff:koff + ksz],
                    qkT[pair][p0:p0 + 64, qoff:qoff + qsz],
                    start=True, stop=True,
                )

            def av_mm(av, h, kt, src, src_off, qsz):
                koff, ksz = KT[kt]
                nc.tensor.matmul(
                    av[0:128, 0:qsz],
                    v_ext[kt][0:ksz, h * VW:h * VW + 128],
                    src[0:ksz, src_off:src_off + qsz],
                    start=(kt == 0), stop=(kt == 8),
                    skip_group_check=True,
                )

            def qkp_chain(ot, qoff, qsz):
                with nc.named_scope("qk_proj"):
                    ps = psP.tile([128, 512], F32, tag="proj", name="ps_proj")
                    for c in range(CT):
                        nc.tensor.matmul(
                            ps[:, :qsz],
                            wqk[c][:, ot * 128:(ot + 1) * 128],
                            xT[c][:, qoff:qoff + qsz],
                            start=(c == 0), stop=(c == CT - 1),
                        )
                    nc.vector.tensor_copy(qkT[ot][:, qoff:qoff + qsz],
                                          ps[:, :qsz])

            def qkp_fillers(pair):
                # fill order: both heads' big chunks first, tails last
                out = []
                for (qoff, qsz) in QC + [QTAIL]:
                    for ot in (pair, 6 + pair):
                        out.append((ot, qoff, qsz))
                return out

            # denominator gather tiles: heads 0-9 (A) and 10-11 (B) split so
            # each batched reciprocal can run as soon as its pairs are done
            dnsA = {qi: p_sm.tile([10, 512], F32, tag=f"dnsA{qi}",
                                  name=f"dnsA{qi}", bufs=1) for qi in range(2)}
            dnsB = {qi: p_sm.tile([2, 512], F32, tag=f"dnsB{qi}",
                                  name=f"dnsB{qi}", bufs=1) for qi in range(2)}
            dnstA = p_sm.tile([10, 8], F32, tag="dnstA", name="dnstA", bufs=1)
            dnstB = p_sm.tile([2, 8], F32, tag="dnstB", name="dnstB", bufs=1)
            parked = {}

            def dns_row(qi, h):
                if qi == "t":
                    return (dnstA, h) if h < 10 else (dnstB, h - 10)
                return (dnsA[qi], h) if h < 10 else (dnsB[qi], h - 10)

            def attn_chunk(pair, qi, fillers):
                qoff, qsz = QC[qi]
                h0 = 2 * pair
                h1 = 2 * pair + 1
                with nc.named_scope("attn"):
                    avs = {h0: psA.tile([128, 512], F32, tag="av", name="ps_av",
                                        bufs=2),
                           h1: psA.tile([128, 512], F32, tag="av", name="ps_av",
                                        bufs=2)}
                    # warmer into the av tile (before its accumulation starts)
                    # keeps the PE HAM clock up without waiting on an sc slot
                    nc.tensor.matmul(
                        avs[h0][:, 0:512],
                        qkT[pair][:, 0:128],
                        qkT[pair][:, 0:512],
                        start=True, stop=True,
                    )
                    for g in SC_GROUPS:
                        sc_slots = {}
                        for hp in range(2):
                            sc_slots[hp] = psA.tile([128, 1024], F32, tag="sc",
                                                    name="ps_sc", bufs=2)
                        for gi, kt in enumerate(g):
                            for hp in range(2):
                                scores_mm(sc_slots[hp], pair, hp, kt,
                                          gi * 512, qoff, qsz)
                        at = {}
                        for hp in range(2):
                            h = 2 * pair + hp
                            a = p_attn.tile([128, 1024], ATTN, tag="attnT",
                                            name="attnT", bufs=8)
                            at[h] = a
                            width = len(g) * 512
                            pmax = max(KT[kt][1] for kt in g)
                            nc.scalar.activation(
                                a[0:pmax, 0:width],
                                sc_slots[hp][0:pmax, 0:width],
                                mybir.ActivationFunctionType.Exp, scale=SCALE,
                            )
                        for gi, kt in enumerate(g):
                            av_mm(avs[h0], h0, kt, at[h0], gi * 512, qsz)
                            av_mm(avs[h1], h1, kt, at[h1], gi * 512, qsz)
                        if fillers:
                            qkp_chain(*fillers.pop(0))
                    for hp in range(2):
                        h = 2 * pair + hp
                        sb = p_sm.tile([VW, 512], F32, tag=f"avsb{qi}_{h}",
                                       name=f"avsb{qi}_{h}", bufs=1)
                        nc.vector.tensor_copy(sb[0:VW, 0:qsz],
                                              avs[h][0:VW, 0:qsz])
                        dt, row = dns_row(qi, h)
                        nc.sync.dma_start(dt[row:row + 1, 0:qsz],
                                          sb[D:VW, 0:qsz])
                        parked[(qi, h)] = sb

            def attn_tail(pair):
                qoff, qsz = QTAIL
                with nc.named_scope("attn"):
                    for hp in range(2):
                        h = 2 * pair + hp
                        sc = psP.tile([128, 512], F32, tag="proj", name="ps_proj")
                        for kt in range(9):
                            scores_mm(sc, pair, hp, kt, kt * 8, qoff, qsz)
                        a = p_attn.tile([128, 1024], ATTN, tag="attnT",
                                        name="attnT", bufs=8)
                        nc.scalar.activation(
                            a[:, 0:64], sc[:, 0:64],
                            mybir.ActivationFunctionType.Exp, scale=SCALE,
                        )
                        nc.scalar.activation(
                            a[0:8, 64:72], sc[0:8, 64:72],
                            mybir.ActivationFunctionType.Exp, scale=SCALE,
                        )
                        av = psP.tile([128, 512], F32, tag="proj", name="ps_proj")
                        for kt in range(9):
                            av_mm(av, h, kt, a, kt * 8, qsz)
                        sb = p_sm.tile([VW, 8], F32, tag=f"avsbt{h}",
                                       name=f"avsbt{h}", bufs=1)
                        nc.vector.tensor_copy(sb[0:VW, 0:qsz], av[0:VW, 0:qsz])
                        dt, row = dns_row("t", h)
                        nc.sync.dma_start(dt[row:row + 1, 0:qsz], sb[D:VW, 0:qsz])
                        parked[("t", h)] = sb

            def norm_part(qi, qoff, qsz, dns_t, heads, rtag):
                # reciprocal (batched over heads) on DVE; broadcast + multiply
                # on gpsimd so the DVE queue never blocks the park copies
                with nc.named_scope("attn"):
                    nh = len(heads)
                    rec = p_sm.tile([nh, 512], F32, tag=rtag, name=rtag, bufs=1)
                    nc.vector.reciprocal(rec[0:nh, 0:qsz], dns_t[0:nh, 0:qsz])
                    for ri, h in enumerate(heads):
                        pair, hp = divmod(h, 2)
                        p0 = 64 * hp
                        rech = p_sm.tile([1, 512], F32, tag="rech", name="rech",
                                         bufs=4)
                        nc.sync.dma_start(rech[0:1, 0:qsz], rec[ri:ri + 1, 0:qsz])
                        bc = p_sm.tile([64, 512], F32, tag="bc", name="bc", bufs=4)
                        nc.gpsimd.partition_broadcast(bc[0:64, 0:qsz],
                                                      rech[0:1, 0:qsz])
                        nc.gpsimd.tensor_mul(
                            aoT[pair][p0:p0 + 64, qoff:qoff + qsz],
                            parked[(qi, h)][0:64, 0:qsz],
                            bc[0:64, 0:qsz],
                        )

            def e_chunk_ot(qoff, qsz, ot):
                with nc.named_scope("out_proj"):
                    ps = psP.tile([128, 512], F32, tag="proj", name="ps_proj")
                    for c in range(CT):
                        nc.tensor.matmul(
                            ps[:, :qsz],
                            wp[c][:, ot * 128:(ot + 1) * 128],
                            aoT[c][:, qoff:qoff + qsz],
                            start=(c == 0), stop=(c == CT - 1),
                        )
                    st = p_stage.tile([128, 512],
                                      F16 if MT == F16 else F32,
                                      tag="ystage", name="ystage")
                    nc.scalar.activation(
                        st[:, :qsz], ps[:, :qsz],
                        mybir.ActivationFunctionType.Identity,
                        bias=bp_sb[ot][:, 0:1], scale=1.0)
                    nc.sync.dma_start(
                        yT_d.ap()[ot * 128:(ot + 1) * 128, qoff:qoff + qsz],
                        st[:, :qsz])

            # pair 0's projection has no previous attention to hide under
            for item in qkp_fillers(0):
                qkp_chain(*item)
            for pair in range(6):
                fillers = qkp_fillers(pair + 1) if pair < 5 else []
                attn_chunk(pair, 0, fillers)
                attn_chunk(pair, 1, fillers)
                for item in fillers:
                    qkp_chain(*item)
                if pair == 4:
                    norm_part(0, QC[0][0], QC[0][1], dnsA[0],
                              list(range(10)), "recA0")
                if pair == 5:
                    norm_part(0, QC[0][0], QC[0][1], dnsB[0], [10, 11], "recB0")
            norm_part(1, QC[1][0], QC[1][1], dnsA[1], list(range(10)), "recA1")
            norm_part(1, QC[1][0], QC[1][1], dnsB[1], [10, 11], "recB1")

            # ---- drain: out-projection qc0 interleaved with the 8-query
            # attention tails (mixed big/small matmuls keep the clock up) ----
            for ot in range(CT):
                e_chunk_ot(QC[0][0], QC[0][1], ot)
                attn_tail(ot)
                if ot == 4:
                    norm_part("t", QTAIL[0], QTAIL[1], dnstA,
                              list(range(10)), "recAt")
            norm_part("t", QTAIL[0], QTAIL[1], dnstB, [10, 11], "recBt")
            for ot in range(CT):
                e_chunk_ot(QC[1][0], QC[1][1], ot)
            for ot in range(CT):
                e_chunk_ot(QTAIL[0], QTAIL[1], ot)

    nc.compile()
    return nc


def _memset(nc, AT, ap, one):
    if AT == BF16:
        nc.vector.memset(ap.bitcast(mybir.dt.uint16), 0x3F80 if one else 0)
    elif AT == F16:
        nc.vector.memset(ap.bitcast(mybir.dt.uint16), 0x3C00 if one else 0)
    else:
        nc.vector.memset(ap.bitcast(mybir.dt.uint32), 0x3F800000 if one else 0)


_NC_CACHE = {}
_MODE = "fp16"
TRACE = False
TRACE_KW = {}
LAST_RES = None


def kernel(x, w_qkv, w_proj, b_proj):
    x = np.asarray(x, np.float32)
    w_qkv = np.asarray(w_qkv, np.float32)
    w_proj = np.asarray(w_proj, np.float32)
    b_proj = np.asarray(b_proj, np.float32)
    B = x.shape[0]
    assert x.shape == (8, NTOK, C), x.shape

    mt = np.float16 if _MODE == "fp16" else np.float32
    wqkT = np.ascontiguousarray(w_qkv[:2 * C].T.astype(mt))
    wvT = np.ascontiguousarray(w_qkv[2 * C:].T.astype(mt))
    wpT = np.ascontiguousarray(w_proj.T.astype(mt))
    bp = np.ascontiguousarray(b_proj.reshape(C, 1))
    in_maps = []
    for b in range(B):
        xT = np.zeros((C, T), mt)
        xT[:, :NTOK] = x[b].T.astype(mt)
        in_maps.append({"xT": xT, "wqkT": wqkT, "wvT": wvT, "wpT": wpT, "bp": bp})

    if _MODE not in _NC_CACHE:
        _NC_CACHE[_MODE] = build(matmul_dtype=_MODE)
    nc = _NC_CACHE[_MODE]
    from concourse import bass_utils
    res = bass_utils.run_bass_kernel_spmd(nc, in_maps, core_ids=list(range(B)),
                                          trace=TRACE, **TRACE_KW)
    global LAST_RES
    LAST_RES = res
    y = np.stack([res.results[b]["yT"][:, :NTOK].T for b in range(B)])
    return np.ascontiguousarray(y.astype(np.float32))


# revision 19
# speedup vs baseline: 1.8386x; 1.8386x over previous
"""Trainium2 Bass kernel: batched multi-head self-attention (nn_Attention).

y = softmax(q k^T / sqrt(64)) v, projected; x (8, 1025, 768), 12 heads x 64.

Strategy: batch-parallel across the 8 NeuronCores (one batch element per
core, no collectives). Per core, everything is kept feature-major
(transposed) so no on-chip transposes are needed:
  qkT = wqkT.T @ xT;  v = xT.T @ wvT (with a per-head ones column);
  scoresT = kT.T @ qT (keys on partitions, head pairs row-packed in the PE
  array);  exp on the scalar engine;  [v|1](128-wide).T @ attnT accumulated
  over key tiles yields the weighted values AND the softmax denominator in
  one PSUM accumulation.

Pipeline: v-projection first (c-outer so the PE starts as soon as the first
x/wv chunks land), then a fused per-head-pair loop [qk-projection for the
pair -> attention for the pair] so the scalar engine's exp stream overlaps
the projection matmuls instead of idling through a separate projection
phase. Normalization is deferred: numerators parked in SBUF, denominators
DMA-gathered onto 12 partitions, ONE batched reciprocal per query chunk
(DVE cost is free-size-only), reciprocal rows DMA-scattered and broadcast,
then multiplied into aoT. Out-projection (+bias on the scalar engine)
drains at the end.

Operands are fp16 (inputs/weights/q/k/v, ~2e-3 relative accuracy) except the
exp'd attention weights, which are bf16 (exp reaches ~5e6, beyond fp16
range); all accumulation is fp32 in PSUM. Full-array "warmer" matmuls keep
the PE HAM clock at 8/8 through the partial-array attention matmuls.
"""
import sys

try:
    import concourse.bass  # noqa: F401
except ImportError:
    sys.path.insert(0, "/opt/trn_rl_repo")

import numpy as np

from contextlib import ExitStack

import concourse.bass as bass
import concourse.tile as tile
from concourse import bacc, mybir

F32 = mybir.dt.float32
F32R = mybir.dt.float32r
BF16 = mybir.dt.bfloat16
F16 = mybir.dt.float16

C = 768
H = 12
D = 64
NTOK = 1025
T = 1032
CT = C // 128
SCALE = D ** -0.5

KT = [(i * 128, 128) for i in range(8)] + [(1024, 8)]
QC = [(0, 512), (512, 512)]
QTAIL = (1024, 8)
SC_GROUPS = [(0, 1), (2, 3), (4, 5), (6, 7), (8,)]
VW = 65


def build(matmul_dtype="fp16"):
    if matmul_dtype == "fp16":
        MT = AT = F16
        ATTN = BF16
    elif matmul_dtype == "bf16":
        MT = AT = ATTN = BF16
    else:
        MT = AT = ATTN = F32
    nc = bacc.Bacc("TRN2", target_bir_lowering=False, debug=False, num_devices=8)

    xT_d = nc.dram_tensor("xT", [C, T], MT, kind="ExternalInput")
    wqkT_d = nc.dram_tensor("wqkT", [C, 2 * C], MT, kind="ExternalInput")
    wvT_d = nc.dram_tensor("wvT", [C, C], MT, kind="ExternalInput")
    wpT_d = nc.dram_tensor("wpT", [C, C], MT, kind="ExternalInput")
    bp_d = nc.dram_tensor("bp", [C, 1], F32, kind="ExternalInput")
    yT_d = nc.dram_tensor("yT", [C, T], F16 if matmul_dtype == "fp16" else F32,
                          kind="ExternalOutput")

    with tile.TileContext(nc) as tc, ExitStack() as ctx:
        p_qk = ctx.enter_context(tc.tile_pool(name="qk", bufs=1))
        p_v = ctx.enter_context(tc.tile_pool(name="v", bufs=1))
        p_ao = ctx.enter_context(tc.tile_pool(name="ao", bufs=1))
        p_bp = ctx.enter_context(tc.tile_pool(name="bp", bufs=1))
        p_attn = ctx.enter_context(tc.tile_pool(name="attn", bufs=1))
        p_sm = ctx.enter_context(tc.tile_pool(name="sm", bufs=1))
        p_stage = ctx.enter_context(tc.tile_pool(name="stage", bufs=4))
        p_w = ctx.enter_context(tc.tile_pool(name="w", bufs=1))

        qkT = [p_qk.tile([128, T], AT, tag=f"qkT{i}", name=f"qkT{i}")
               for i in range(12)]
        v_ext = [p_v.tile([128, H * VW + 63], AT, tag=f"v{i}", name=f"v{i}")
                 for i in range(9)]
        aoT = [p_ao.tile([128, T], MT, tag=f"ao{i}", name=f"ao{i}")
               for i in range(CT)]
        bp_sb = [p_bp.tile([128, 1], F32, tag=f"bp{i}", name=f"bp{i}")
                 for i in range(CT)]

        xT = [p_w.tile([128, T], MT, tag=f"x{i}", name=f"x{i}") for i in range(CT)]
        wvT = [p_w.tile([128, C], MT, tag=f"wv{i}", name=f"wv{i}")
               for i in range(CT)]
        wqk = [p_w.tile([128, 2 * C], MT, tag=f"wqk{i}", name=f"wqk{i}")
               for i in range(CT)]
        wp = [p_w.tile([128, C], MT, tag=f"wp{i}", name=f"wp{i}")
              for i in range(CT)]
        for c in range(CT):
            nc.sync.dma_start(xT[c][:], xT_d.ap()[c * 128:(c + 1) * 128, :])
            nc.sync.dma_start(wvT[c][:], wvT_d.ap()[c * 128:(c + 1) * 128, :])
        for c in range(CT):
            nc.sync.dma_start(wqk[c][:], wqkT_d.ap()[c * 128:(c + 1) * 128, :])
        for c in range(CT):
            nc.sync.dma_start(wp[c][:], wpT_d.ap()[c * 128:(c + 1) * 128, :])
            nc.sync.dma_start(bp_sb[c][:], bp_d.ap()[c * 128:(c + 1) * 128, :])

        # ---- v projection: c-outer in nt-blocks of 4 so the first matmul
        # only waits on the first x/wv chunk DMA ----
        with nc.named_scope("v_proj"), \
             tc.tile_pool(name="psv", bufs=4, space="PSUM") as psv:
            for blk in ((0, 1, 2, 3), (4, 5, 6, 7), (8,)):
                pss = {nt: psv.tile([128, 768], F32, tag="psv", name="psv")
                       for nt in blk}
                for c in range(CT):
                    for nt in blk:
                        noff, nsz = KT[nt]
                        for (voff, vsz) in ((0, 512), (512, 256)):
                            nc.tensor.matmul(
                                pss[nt][:nsz, voff:voff + vsz],
                                xT[c][:, noff:noff + nsz],
                                wvT[c][:, voff:voff + vsz],
                                start=(c == 0), stop=(c == CT - 1),
                            )
                for nt in blk:
                    noff, nsz = KT[nt]
                    for (voff, vsz) in ((0, 512), (512, 256)):
                        nh = vsz // D
                        h0 = voff // D
                        dst = (
                            v_ext[nt][0:nsz, h0 * VW:(h0 + nh) * VW]
                            .rearrange("p (hh w) -> p hh w", w=VW)[:, :, 0:D]
                        )
                        src = pss[nt][0:nsz, voff:voff + vsz].rearrange(
                            "p (hh w) -> p hh w", w=D)
                        nc.vector.tensor_copy(dst, src)
                    if nt < 8:
                        ones_col = (
                            v_ext[nt][0:nsz, 0:H * VW]
                            .rearrange("p (hh w) -> p hh w", w=VW)[:, :, D:VW]
                        )
                        _memset(nc, AT, ones_col, one=True)
                    else:
                        pad_col = (
                            v_ext[nt][0:nsz, 0:H * VW]
                            .rearrange("p (hh w) -> p hh w", w=VW)[:, :, D:VW]
                        )
                        _memset(nc, AT, pad_col, one=False)
                        one_row = (
                            v_ext[nt][0:1, 0:H * VW]
                            .rearrange("p (hh w) -> p hh w", w=VW)[:, :, D:VW]
                        )
                        _memset(nc, AT, one_row, one=True)
            for nt in range(9):
                _memset(nc, AT, v_ext[nt][:, H * VW:H * VW + 63], one=False)

        # ---- fused pair loop: qk-projection for the pair, then attention ----
        with tc.tile_pool(name="psP", bufs=2, space="PSUM") as psP, \
             tc.tile_pool(name="psA", bufs=1, space="PSUM") as psA:

            def scores_mm(sc, pair, h_in_pair, kt, dst_off, qoff, qsz):
                koff, ksz = KT[kt]
                p0 = 64 * h_in_pair
                nc.tensor.matmul(
                    sc[0:ksz, dst_off:dst_off + qsz],
                    qkT[6 + pair][p0:p0 + 64, koff:koff + ksz],
                    qkT[pair][p0:p0 + 64, qoff:qoff + qsz],
                    start=True, stop=True,
                )

            def av_mm(av, h, kt, src, src_off, qsz):
                koff, ksz = KT[kt]
                nc.tensor.matmul(
                    av[0:128, 0:qsz],
                    v_ext[kt][0:ksz, h * VW:h * VW + 128],
                    src[0:ksz, src_off:src_off + qsz],
                    start=(kt == 0), stop=(kt == 8),
                    skip_group_check=True,
                )

            def qkp_chain(ot, qoff, qsz):
                with nc.named_scope("qk_proj"):
                    ps = psP.tile([128, 512], F32, tag="proj", name="ps_proj")
                    for c in range(CT):
                        nc.tensor.matmul(
                            ps[:, :qsz],
                            wqk[c][:, ot * 128:(ot + 1) * 128],
                            xT[c][:, qoff:qoff + qsz],
                            start=(c == 0), stop=(c == CT - 1),
                        )
                    nc.vector.tensor_copy(qkT[ot][:, qoff:qoff + qsz],
                                          ps[:, :qsz])

            def qkp_fillers(pair):
                # fill order: both heads' big chunks first, tails last
                out = []
                for (qoff, qsz) in QC + [QTAIL]:
                    for ot in (pair, 6 + pair):
                        out.append((ot, qoff, qsz))
                return out

            # denominator gather tiles: all 12 heads' denom rows collected by
            # DMA onto 12 partitions so ONE reciprocal serves the chunk
            dns = {qi: p_sm.tile([12, 512], F32, tag=f"dns{qi}",
                                 name=f"dns{qi}", bufs=1) for qi in range(2)}
            dnst = p_sm.tile([12, 8], F32, tag="dnst", name="dnst", bufs=1)
            parked = {}

            def dns_row(qi, h):
                return (dnst, h) if qi == "t" else (dns[qi], h)

            def attn_chunk(pair, qi, fillers):
                qoff, qsz = QC[qi]
                h0 = 2 * pair
                h1 = 2 * pair + 1
                with nc.named_scope("attn"):
                    avs = {h0: psA.tile([128, 512], F32, tag="av", name="ps_av",
                                        bufs=2),
                           h1: psA.tile([128, 512], F32, tag="av", name="ps_av",
                                        bufs=2)}
                    # warmer into the av tile (before its accumulation starts)
                    # keeps the PE HAM clock up without waiting on an sc slot
                    nc.tensor.matmul(
                        avs[h0][:, 0:512],
                        qkT[pair][:, 0:128],
                        qkT[pair][:, 0:512],
                        start=True, stop=True,
                    )
                    for g in SC_GROUPS:
                        sc_slots = {}
                        for hp in range(2):
                            sc_slots[hp] = psA.tile([128, 1024], F32, tag="sc",
                                                    name="ps_sc", bufs=2)
                        for gi, kt in enumerate(g):
                            for hp in range(2):
                                scores_mm(sc_slots[hp], pair, hp, kt,
                                          gi * 512, qoff, qsz)
                        at = {}
                        for hp in range(2):
                            h = 2 * pair + hp
                            a = p_attn.tile([128, 1024], ATTN, tag="attnT",
                                            name="attnT", bufs=8)
                            at[h] = a
                            width = len(g) * 512
                            pmax = max(KT[kt][1] for kt in g)
                            nc.scalar.activation(
                                a[0:pmax, 0:width],
                                sc_slots[hp][0:pmax, 0:width],
                                mybir.ActivationFunctionType.Exp, scale=SCALE,
                            )
                        for gi, kt in enumerate(g):
                            av_mm(avs[h0], h0, kt, at[h0], gi * 512, qsz)
                            av_mm(avs[h1], h1, kt, at[h1], gi * 512, qsz)
                        if fillers:
                            qkp_chain(*fillers.pop(0))
                    for hp in range(2):
                        h = 2 * pair + hp
                        sb = p_sm.tile([VW, 512], F32, tag=f"avsb{qi}_{h}",
                                       name=f"avsb{qi}_{h}", bufs=1)
                        nc.vector.tensor_copy(sb[0:VW, 0:qsz],
                                              avs[h][0:VW, 0:qsz])
                        dt, row = dns_row(qi, h)
                        nc.sync.dma_start(dt[row:row + 1, 0:qsz],
                                          sb[D:VW, 0:qsz])
                        parked[(qi, h)] = sb

            def attn_tail(pair):
                qoff, qsz = QTAIL
                with nc.named_scope("attn"):
                    for hp in range(2):
                        h = 2 * pair + hp
                        sc = psP.tile([128, 512], F32, tag="proj", name="ps_proj")
                        for kt in range(9):
                            scores_mm(sc, pair, hp, kt, kt * 8, qoff, qsz)
                        a = p_attn.tile([128, 1024], ATTN, tag="attnT",
                                        name="attnT", bufs=8)
                        nc.scalar.activation(
                            a[:, 0:64], sc[:, 0:64],
                            mybir.ActivationFunctionType.Exp, scale=SCALE,
                        )
                        nc.scalar.activation(
                            a[0:8, 64:72], sc[0:8, 64:72],
                            mybir.ActivationFunctionType.Exp, scale=SCALE,
                        )
                        av = psP.tile([128, 512], F32, tag="proj", name="ps_proj")
                        for kt in range(9):
                            av_mm(av, h, kt, a, kt * 8, qsz)
                        sb = p_sm.tile([VW, 8], F32, tag=f"avsbt{h}",
                                       name=f"avsbt{h}", bufs=1)
                        nc.vector.tensor_copy(sb[0:VW, 0:qsz], av[0:VW, 0:qsz])
                        dt, row = dns_row("t", h)
                        nc.sync.dma_start(dt[row:row + 1, 0:qsz], sb[D:VW, 0:qsz])
                        parked[("t", h)] = sb

            def norm_chunk(qi, qoff, qsz, dns_t, rtag):
                # one batched reciprocal per chunk; reciprocal rows fanned out
                # to 64 partitions by replicating DMA (0-stride free dim)
                with nc.named_scope("attn"):
                    rec = p_sm.tile([12, 512], F32, tag=rtag, name=rtag, bufs=1)
                    nc.vector.reciprocal(rec[0:12, 0:qsz], dns_t[0:12, 0:qsz])
                    for h in range(12):
                        pair, hp = divmod(h, 2)
                        p0 = 64 * hp
                        bc = p_sm.tile([64, 512], F32, tag="bc", name="bc", bufs=4)
                        nc.sync.dma_start(
                            bc[0:64, 0:qsz],
                            rec[h:h + 1, 0:qsz].unsqueeze(1)
                            .broadcast_to((1, 64, qsz)))
                        nc.vector.tensor_mul(
                            aoT[pair][p0:p0 + 64, qoff:qoff + qsz],
                            parked[(qi, h)][0:64, 0:qsz],
                            bc[0:64, 0:qsz],
                        )

            def e_chunk_ot(qoff, qsz, ot):
                with nc.named_scope("out_proj"):
                    ps = psP.tile([128, 512], F32, tag="proj", name="ps_proj")
                    for c in range(CT):
                        nc.tensor.matmul(
                            ps[:, :qsz],
                            wp[c][:, ot * 128:(ot + 1) * 128],
                            aoT[c][:, qoff:qoff + qsz],
                            start=(c == 0), stop=(c == CT - 1),
                        )
                    st = p_stage.tile([128, 512],
                                      F16 if MT == F16 else F32,
                                      tag="ystage", name="ystage")
                    nc.scalar.activation(
                        st[:, :qsz], ps[:, :qsz],
                        mybir.ActivationFunctionType.Identity,
                        bias=bp_sb[ot][:, 0:1], scale=1.0)
                    nc.sync.dma_start(
                        yT_d.ap()[ot * 128:(ot + 1) * 128, qoff:qoff + qsz],
                        st[:, :qsz])

            # pair 0's projection has no previous attention to hide under
            for item in qkp_fillers(0):
                qkp_chain(*item)
            for pair in range(6):
                fillers = qkp_fillers(pair + 1) if pair < 5 else []
                attn_chunk(pair, 0, fillers)
                if pair == 5:
                    # qc0 norm in pair 5's qc1 window: its only DVE
                    # competitors then are pair 5's own final parks
                    norm_chunk(0, QC[0][0], QC[0][1], dns[0], "rec0")
                attn_chunk(pair, 1, fillers)
                for item in fillers:
                    qkp_chain(*item)
            norm_chunk(1, QC[1][0], QC[1][1], dns[1], "rec1")

            # ---- drain: out-projection qc0 interleaved with the 8-query
            # attention tails (mixed big/small matmuls keep the clock up) ----
            for ot in range(CT):
                e_chunk_ot(QC[0][0], QC[0][1], ot)
                attn_tail(ot)
            norm_chunk("t", QTAIL[0], QTAIL[1], dnst, "rect")
            for ot in range(CT):
                e_chunk_ot(QC[1][0], QC[1][1], ot)
            for ot in range(CT):
                e_chunk_ot(QTAIL[0], QTAIL[1], ot)

    nc.compile()
    return nc


def _memset(nc, AT, ap, one):
    if AT == BF16:
        nc.vector.memset(ap.bitcast(mybir.dt.uint16), 0x3F80 if one else 0)
    elif AT == F16:
        nc.vector.memset(ap.bitcast(mybir.dt.uint16), 0x3C00 if one else 0)
    else:
        nc.vector.memset(ap.bitcast(mybir.dt.uint32), 0x3F800000 if one else 0)


_NC_CACHE = {}
_MODE = "fp16"
TRACE = False
TRACE_KW = {}
LAST_RES = None


def kernel(x, w_qkv, w_proj, b_proj):
    x = np.asarray(x, np.float32)
    w_qkv = np.asarray(w_qkv, np.float32)
    w_proj = np.asarray(w_proj, np.float32)
    b_proj = np.asarray(b_proj, np.float32)
    B = x.shape[0]
    assert x.shape == (8, NTOK, C), x.shape

    mt = np.float16 if _MODE == "fp16" else np.float32
    wqkT = np.ascontiguousarray(w_qkv[:2 * C].T.astype(mt))
    wvT = np.ascontiguousarray(w_qkv[2 * C:].T.astype(mt))
    wpT = np.ascontiguousarray(w_proj.T.astype(mt))
    bp = np.ascontiguousarray(b_proj.reshape(C, 1))
    in_maps = []
    for b in range(B):
        xT = np.zeros((C, T), mt)
        xT[:, :NTOK] = x[b].T.astype(mt)
        in_maps.append({"xT": xT, "wqkT": wqkT, "wvT": wvT, "wpT": wpT, "bp": bp})

    if _MODE not in _NC_CACHE:
        _NC_CACHE[_MODE] = build(matmul_dtype=_MODE)
    nc = _NC_CACHE[_MODE]
    from concourse import bass_utils
    res = bass_utils.run_bass_kernel_spmd(nc, in_maps, core_ids=list(range(B)),
                                          trace=TRACE, **TRACE_KW)
    global LAST_RES
    LAST_RES = res
    y = np.stack([res.results[b]["yT"][:, :NTOK].T for b in range(B)])
    return np.ascontiguousarray(y.astype(np.float32))
